# revision 18
# baseline (speedup 1.0000x reference)
"""Multi-head attention (B=4, S=2048, D=1024, H=16, E=64) on 8 TRN2 NeuronCores.

Sharding: core c handles batch b=c//2 and query-half qh=c%2 (1024 query tokens).
K/V are computed per-core for the full 2048-token sequence of its batch (2x
duplicated K/V projection work, but zero collectives / zero cross-core deps).

v2 (vs fp32r baseline at 868us):
  - all PE matmuls in fp16 (1 cyc/row vs fp32_mode=HIGH's ~2, FWL enabled,
    and roughly half the PE energy -> less HAM duty-cycle throttling)
  - scores matmuls for the 2 heads of a pass are row-tiled (head A in PE rows
    0-63, head B in rows 64-127 via tile_position) and issued adjacently so
    they run concurrently in the array
  - exp computed as exp(s/8 - 4) so fp16 can't overflow (max observed |s/8|
    ~7.6; the constant shift cancels in the softmax normalization)
  - V kept fully resident in SBUF (no DRAM spill round-trip)
  - V/out-proj biases applied on DVE from PE-broadcast bias tiles (no
    ones-row matmuls in inner loops)

Per-core program (SPMD, identical on all cores):
  phase 0: V = x @ wv + bv for all 16 heads -> v_all[tok128, chunk, head, 65]
           with a ones-column at [...,64] (softmax sums ride along att@V).
  passes p=0..7 (heads 2p, 2p+1):
    KT[128he, 2048tok] = (wk_p.T @ xT) + bk, QT[128he, 1024tq] likewise
    per tq-tile of 512: per chunk-pair g: scoresT = KT.T-slices @ QT (A,B
    row-tiled), exp on ScalarE from PSUM, attT[65,512] += [V|1].T @ exp
    normalize: recip(sum row) -> ones-matmul broadcast -> DVE multiply
  phase 2: out[tok,1024] = att @ wo.T + bo (bias via DVE add)
"""

import os

import numpy as np

import concourse.bass as bass
import concourse.mybir as mybir
import concourse.tile as tile
from concourse import bacc
from concourse.bass_utils import run_bass_kernel_spmd

FP32 = mybir.dt.float32
FP16 = mybir.dt.float16
AF = mybir.ActivationFunctionType

B, S, D, H, E = 4, 2048, 1024, 16, 64
NCORES = 8
TQ = S // 2  # query tokens per core
SCALE = 1.0 / float(np.sqrt(E))
# exp(s*SCALE + EXP_BIAS): the shift cancels in softmax normalization and
# keeps fp16 exp finite up to s*SCALE = 19 (observed max over this input
# distribution is ~15.2)
EXP_BIAS = -8.0

_CACHE = {}
DEBUG = bool(int(os.environ.get("KDBG", "0")))


def build_nc():
    nc = bacc.Bacc("TRN2", target_bir_lowering=False)

    xT = nc.dram_tensor("xT", [D, S], FP16, kind="ExternalInput")
    xTq = nc.dram_tensor("xTq", [D, TQ], FP16, kind="ExternalInput")
    wq_t = nc.dram_tensor("wq_t", [D, H * E], FP16, kind="ExternalInput")
    wk_t = nc.dram_tensor("wk_t", [D, H * E], FP16, kind="ExternalInput")
    wv_t = nc.dram_tensor("wv_t", [D, H * E], FP16, kind="ExternalInput")
    wo_t = nc.dram_tensor("wo_t", [D, D], FP16, kind="ExternalInput")
    bqp = nc.dram_tensor("bqp", [128, 8], FP32, kind="ExternalInput")
    bkp = nc.dram_tensor("bkp", [128, 8], FP32, kind="ExternalInput")
    bv_row = nc.dram_tensor("bv_row", [1, H * E], FP16, kind="ExternalInput")
    bo_row = nc.dram_tensor("bo_row", [1, D], FP16, kind="ExternalInput")
    out = nc.dram_tensor("out", [TQ, D], FP32, kind="ExternalOutput")
    if DEBUG:
        dbg_v = nc.dram_tensor("dbg_v", [128, 16, H, E + 1], FP16, kind="ExternalOutput")
        dbg_kt = nc.dram_tensor("dbg_kt", [128, S], FP16, kind="ExternalOutput")
        dbg_qt = nc.dram_tensor("dbg_qt", [128, TQ], FP16, kind="ExternalOutput")
        dbg_exp = nc.dram_tensor("dbg_exp", [128, 2, 512], FP16, kind="ExternalOutput")
        dbg_rec = nc.dram_tensor("dbg_rec", [1, 512], FP32, kind="ExternalOutput")
        dbg_att = nc.dram_tensor("dbg_att", [128, TQ], FP16, kind="ExternalOutput")

    xT_r = xT.rearrange("(t p) s -> p t s", p=128)  # [128, 8, 2048]
    xTq_r = xTq.rearrange("(t p) s -> p t s", p=128)  # [128, 8, 1024]
    wq_r = wq_t.rearrange("(t p) m -> p t m", p=128)  # [128, 8, 1024]
    wk_r = wk_t.rearrange("(t p) m -> p t m", p=128)
    wv_r = wv_t.rearrange("(t p) m -> p t m", p=128)
    wo_r = wo_t.rearrange("(t p) m -> p t m", p=128)

    with tile.TileContext(nc) as tc:
        with (
            tc.tile_pool(name="xt", bufs=1) as xt_pool,
            tc.tile_pool(name="wkq", bufs=2) as wkq_pool,
            tc.tile_pool(name="wv", bufs=2) as wv_pool,
            tc.tile_pool(name="wo", bufs=1) as wo_pool,
            tc.tile_pool(name="kt", bufs=2) as kt_pool,
            tc.tile_pool(name="qt", bufs=2) as qt_pool,
            tc.tile_pool(name="vall", bufs=1) as vall_pool,
            tc.tile_pool(name="expp", bufs=4) as exp_pool,
            tc.tile_pool(name="attT", bufs=8) as attT_pool,
            tc.tile_pool(name="small", bufs=2) as small_pool,
            tc.tile_pool(name="ones", bufs=1) as ones_pool,
            tc.tile_pool(name="ps_s", bufs=2, space="PSUM") as ps_scores,
            tc.tile_pool(name="ps_a", bufs=1, space="PSUM") as ps_att,
            tc.tile_pool(name="ps_sig", bufs=1, space="PSUM") as ps_sig,
            tc.tile_pool(name="ps_g", bufs=2, space="PSUM") as ps_gen,
        ):
            # ---- persistent tiles ----
            # DMA order matters: tiny bias rows + the first wv half go first
            # so the PE's broadcast matmuls and V-projection aren't stuck
            # behind the 6MB x upload in the queue.
            xt_sb = xt_pool.tile([128, 8, S], FP16, tag="xt")  # 32KB/part
            xtq_sb = xt_pool.tile([128, 8, TQ], FP16, tag="xtq")  # 16KB/part

            ones_row_f = ones_pool.tile([1, 128], FP32, tag="onesrf")
            nc.vector.memset(ones_row_f, 1.0)
            ones_sb = ones_pool.tile([1, 128], FP16, tag="ones")
            with nc.allow_low_precision(reason="ones constant"):
                nc.vector.tensor_copy(out=ones_sb, in_=ones_row_f)
            # sigma weights: [128, 64] with col 0 = 1, rest 0 -> a (128,64)
            # col-tile matmul whose out row 0 is the softmax denominator;
            # sharing the att matmuls' tile mode avoids extra PE mode switches
            onescol_sb = ones_pool.tile([128, 64], FP16, tag="onescol")
            with nc.allow_low_precision(reason="constant"):
                nc.vector.memset(onescol_sb, 0.0)
                nc.vector.memset(onescol_sb[:, 0:1], 1.0)
            bq_sb = ones_pool.tile([128, 8], FP32, tag="bq")
            bk_sb = ones_pool.tile([128, 8], FP32, tag="bk")
            nc.sync.dma_start(out=bq_sb, in_=bqp[:, :])
            nc.sync.dma_start(out=bk_sb, in_=bkp[:, :])
            ebias_sb = ones_pool.tile([128, 1], FP32, tag="ebias")
            nc.vector.memset(ebias_sb, EXP_BIAS)
            bv_sb = ones_pool.tile([1, H * E], FP16, tag="bv")
            bo_sb = ones_pool.tile([1, D], FP16, tag="bo")
            nc.sync.dma_start(out=bv_sb, in_=bv_row[:, :])
            nc.sync.dma_start(out=bo_sb, in_=bo_row[:, :])

            wv_sb0 = wv_pool.tile([128, 8, 512], FP16, tag="wv")
            nc.sync.dma_start(out=wv_sb0, in_=wv_r[:, :, 0:512])
            # split x upload so the first V-proj tiles start sooner
            for sc in range(4):
                nc.sync.dma_start(
                    out=xt_sb[:, :, sc * 512 : (sc + 1) * 512],
                    in_=xT_r[:, :, sc * 512 : (sc + 1) * 512],
                )
            nc.sync.dma_start(out=xtq_sb, in_=xTq_r)
            wv_sb1 = wv_pool.tile([128, 8, 512], FP16, tag="wv")
            nc.sync.dma_start(out=wv_sb1, in_=wv_r[:, :, 512:1024])

            # broadcast bias rows to all 128 partitions (once, via PE ones-matmul)
            bv_bcast = ones_pool.tile([128, H * E], FP32, tag="bvb")  # 4KB
            bo_bcast = ones_pool.tile([128, D], FP32, tag="bob")  # 4KB
            for nt in range(2):
                ps = ps_gen.tile([128, 512], FP32, tag="gen")
                nc.tensor.matmul(
                    out=ps,
                    lhsT=ones_sb[:, :128],
                    rhs=bv_sb[:, nt * 512 : (nt + 1) * 512],
                    start=True,
                    stop=True,
                )
                nc.vector.tensor_copy(
                    out=bv_bcast[:, nt * 512 : (nt + 1) * 512], in_=ps
                )
                ps2 = ps_gen.tile([128, 512], FP32, tag="gen")
                nc.tensor.matmul(
                    out=ps2,
                    lhsT=ones_sb[:, :128],
                    rhs=bo_sb[:, nt * 512 : (nt + 1) * 512],
                    start=True,
                    stop=True,
                )
                nc.vector.tensor_copy(
                    out=bo_bcast[:, nt * 512 : (nt + 1) * 512], in_=ps2
                )

            # V resident in SBUF: [tok128, chunk, head, E]
            v_all = vall_pool.tile([128, 16, H, E], FP16, tag="vall")  # 32KB


            attT_tiles = [
                attT_pool.tile([128, TQ], FP16, tag="attT", name=f"attT{i}")
                for i in range(8)
            ]

            # ---- helpers: PE work groups (emitted inline or as attention fillers) ----
            def v_group(wv_sb, nt, tokt):
                def go():
                    ps = ps_gen.tile([128, 512], FP32, tag="gen")
                    for k in range(8):
                        nc.tensor.matmul(
                            out=ps,
                            lhsT=xt_sb[:, k, tokt * 128 : (tokt + 1) * 128],
                            rhs=wv_sb[:, k, :],
                            start=(k == 0),
                            stop=(k == 7),
                        )
                    with nc.allow_low_precision(reason="V to fp16"):
                        nc.vector.tensor_add(
                            out=v_all[:, tokt, nt * 8 : (nt + 1) * 8, :],
                            in0=ps.rearrange("p (h e) -> p h e", e=E),
                            in1=bv_bcast[
                                :, nt * 512 : (nt + 1) * 512
                            ].rearrange("p (h e) -> p h e", e=E),
                        )
                return go

            def kt_group(wk_sb, kt_sb, p, ts):
                def go():
                    ps = ps_gen.tile([128, 512], FP32, tag="gen")
                    for k in range(8):
                        nc.tensor.matmul(
                            out=ps,
                            lhsT=wk_sb[:, k, :],
                            rhs=xt_sb[:, k, ts * 512 : (ts + 1) * 512],
                            start=(k == 0),
                            stop=(k == 7),
                        )
                    with nc.allow_low_precision(reason="K to fp16"):
                        nc.vector.tensor_scalar_add(
                            out=kt_sb[:, ts * 512 : (ts + 1) * 512],
                            in0=ps,
                            scalar1=bk_sb[:, p : p + 1],
                        )
                return go

            def qt_group(wq_sb, qt_sb, p, qs):
                def go():
                    ps = ps_gen.tile([128, 512], FP32, tag="gen")
                    for k in range(8):
                        nc.tensor.matmul(
                            out=ps,
                            lhsT=wq_sb[:, k, :],
                            rhs=xtq_sb[:, k, qs * 512 : (qs + 1) * 512],
                            start=(k == 0),
                            stop=(k == 7),
                        )
                    with nc.allow_low_precision(reason="Q to fp16"):
                        nc.vector.tensor_scalar_add(
                            out=qt_sb[:, qs * 512 : (qs + 1) * 512],
                            in0=ps,
                            scalar1=bq_sb[:, p : p + 1],
                        )
                return go

            def ktqt_groups(p):
                """DMA wk/wq for pass p, allocate kt/qt tiles, return proj groups."""
                wk_sb = wkq_pool.tile([128, 8, 128], FP16, tag="wk")
                wq_sb = wkq_pool.tile([128, 8, 128], FP16, tag="wq")
                nc.sync.dma_start(out=wk_sb, in_=wk_r[:, :, p * 128 : (p + 1) * 128])
                nc.sync.dma_start(out=wq_sb, in_=wq_r[:, :, p * 128 : (p + 1) * 128])
                kt_sb = kt_pool.tile([128, S], FP16, tag="kt")
                qt_sb = qt_pool.tile([128, TQ], FP16, tag="qt")
                groups = [kt_group(wk_sb, kt_sb, p, ts) for ts in range(4)]
                groups += [qt_group(wq_sb, qt_sb, p, qs) for qs in range(2)]
                return kt_sb, qt_sb, groups

            # ---- phase 0: V projection nt=0 + pass-0 K/Q, emitted densely ----
            for tokt in range(16):
                v_group(wv_sb0, 0, tokt)()
            kt_sb, qt_sb, g0 = ktqt_groups(0)
            for g in g0:
                g()

            # Fillers keep the PE fed during the ScalarE-paced attention loop.
            # must_do: next pass's K/Q projections (deadline = end of this
            # pass); extra: nt=1 V groups (heads 8-15, deadline = pass 4).
            must_do = []
            extra = [v_group(wv_sb1, 1, tokt) for tokt in range(16)]

            def pop_filler(reserve=0):
                if len(must_do) > reserve:
                    must_do.pop(0)()
                elif extra:
                    extra.pop(0)()

            # ---- passes: 2 heads each (A at partitions 0:64, B at 64:128) ----
            wo_sb = wo_pool.tile([128, 16, 512], FP16, tag="wo")  # 16KB
            for p in range(8):
                if p == 6:
                    for nt in range(2):
                        nc.sync.dma_start(
                            out=wo_sb[:, nt * 8 : (nt + 1) * 8, :],
                            in_=wo_r[:, :, nt * 512 : (nt + 1) * 512],
                        )
                if p < 7:
                    kt_next, qt_next, gnext = ktqt_groups(p + 1)
                    must_do.extend(gnext)
                if p == 4:
                    while extra:  # all V must exist before heads 8-15 attend
                        extra.pop(0)()

                if DEBUG and p == 0:
                    nc.sync.dma_start(out=dbg_kt[:, :], in_=kt_sb)
                    nc.sync.dma_start(out=dbg_qt[:, :], in_=qt_sb)

                hA, hB = 2 * p, 2 * p + 1
                for tqt in range(2):
                    # one PSUM bank holds both heads' att accumulators (A in
                    # partitions 0:64 via col-tile (0,0), B in 64:128 via
                    # (0,64)); another holds both softmax sums (rows 0 / 64)
                    att_ab = ps_att.tile([128, 512], FP32, tag="att")
                    sig_ps = ps_sig.tile([128, 512], FP32, tag="sig")
                    for t in range(16):
                        # one 2-bank PSUM tile holds chunk t's scores for BOTH
                        # heads; the pair runs concurrently in disjoint PE row
                        # groups, and one exp instruction covers both
                        ps_s = ps_scores.tile([128, 2, 512], FP32, tag="sc")
                        nc.tensor.matmul(
                            out=ps_s[:, 0, :],
                            lhsT=kt_sb[0:64, t * 128 : (t + 1) * 128],
                            rhs=qt_sb[0:64, tqt * 512 : (tqt + 1) * 512],
                            start=True,
                            stop=True,
                            tile_position=(0, 0),
                        )
                        nc.tensor.matmul(
                            out=ps_s[:, 1, :],
                            lhsT=kt_sb[64:128, t * 128 : (t + 1) * 128],
                            rhs=qt_sb[64:128, tqt * 512 : (tqt + 1) * 512],
                            start=True,
                            stop=True,
                            tile_position=(64, 0),
                        )
                        exp_t = exp_pool.tile([128, 2, 512], FP16, tag="exp")
                        nc.scalar.activation(
                            out=exp_t, in_=ps_s, func=AF.Exp, scale=SCALE,
                            bias=ebias_sb[:, :],
                        )
                        if DEBUG and p == 0 and tqt == 0 and t == 0:
                            nc.sync.dma_start(out=dbg_exp[:, 0, :], in_=exp_t[:, 0, :])
                        if DEBUG and p == 0 and tqt == 0 and t == 1:
                            nc.sync.dma_start(out=dbg_exp[:, 1, :], in_=exp_t[:, 0, :])
                        nc.tensor.matmul(
                            out=att_ab[0:64, :],
                            lhsT=v_all[:, t, hA, :],
                            rhs=exp_t[:, 0, :],
                            start=(t == 0),
                            stop=(t == 15),
                            tile_position=(0, 0),
                        )
                        nc.tensor.matmul(
                            out=att_ab[64:128, :],
                            lhsT=v_all[:, t, hB, :],
                            rhs=exp_t[:, 1, :],
                            start=(t == 0),
                            stop=(t == 15),
                            tile_position=(0, 64),
                        )
                        nc.tensor.matmul(
                            out=sig_ps[0:64, :],
                            lhsT=onescol_sb,
                            rhs=exp_t[:, 0, :],
                            start=(t == 0),
                            stop=(t == 15),
                            tile_position=(0, 0),
                        )
                        nc.tensor.matmul(
                            out=sig_ps[64:128, :],
                            lhsT=onescol_sb,
                            rhs=exp_t[:, 1, :],
                            start=(t == 0),
                            stop=(t == 15),
                            tile_position=(0, 64),
                        )
                        # keep the PE fed while ScalarE crunches exp
                        if t in (3, 7, 11):
                            pop_filler(reserve=2)
                    # tail fillers: occupy the PE while ScalarE drains the
                    # last exp chunks and DVE computes the softmax recip, so
                    # the rb matmuls / next-tqt scores aren't head-of-line
                    # blocked behind an idle wait
                    pop_filler()
                    pop_filler()
                    recA = small_pool.tile([1, 512], FP32, tag="rec32", bufs=2)
                    recB = small_pool.tile([1, 512], FP32, tag="rec32b", bufs=2)
                    nc.vector.reciprocal(out=recA, in_=sig_ps[0:1, :])
                    nc.vector.reciprocal(out=recB, in_=sig_ps[64:65, :])
                    if DEBUG and p == 0 and tqt == 0:
                        nc.sync.dma_start(out=dbg_rec[:, :], in_=recA)
                    rA16 = small_pool.tile([1, 512], FP16, tag="recr", bufs=2)
                    rB16 = small_pool.tile([1, 512], FP16, tag="recrb", bufs=2)
                    with nc.allow_low_precision(reason="softmax recip"):
                        nc.vector.tensor_copy(out=rA16, in_=recA)
                        nc.vector.tensor_copy(out=rB16, in_=recB)
                    rb_ps = ps_gen.tile([128, 512], FP32, tag="gen")
                    nc.tensor.matmul(
                        out=rb_ps[0:64, :],
                        lhsT=ones_sb[:, :64],
                        rhs=rA16,
                        start=True,
                        stop=True,
                        tile_position=(0, 0),
                    )
                    nc.tensor.matmul(
                        out=rb_ps[64:128, :],
                        lhsT=ones_sb[:, :64],
                        rhs=rB16,
                        start=True,
                        stop=True,
                        tile_position=(0, 64),
                    )
                    rb_sb = small_pool.tile([128, 512], FP32, tag="stg", bufs=2)
                    nc.vector.tensor_copy(out=rb_sb, in_=rb_ps)
                    with nc.allow_low_precision(reason="attT fp16"):
                        nc.vector.tensor_mul(
                            out=attT_tiles[p][:, tqt * 512 : (tqt + 1) * 512],
                            in0=att_ab,
                            in1=rb_sb,
                        )

                # next pass's K/Q must be fully emitted before its attention
                while must_do:
                    must_do.pop(0)()
                if p < 7:
                    kt_sb, qt_sb = kt_next, qt_next

            if DEBUG:
                nc.sync.dma_start(out=dbg_att[:, :], in_=attT_tiles[0])

            # ---- phase 2: output projection (wo was prefetched in pass 6) ----
            for tokt in range(8):
                for nt in range(2):
                    ps = ps_gen.tile([128, 512], FP32, tag="gen")
                    for t in range(8):
                        nc.tensor.matmul(
                            out=ps,
                            lhsT=attT_tiles[t][:, tokt * 128 : (tokt + 1) * 128],
                            rhs=wo_sb[:, nt * 8 + t, :],
                            start=(t == 0),
                            stop=(t == 7),
                        )
                    ostg = small_pool.tile([128, 512], FP32, tag="stg", bufs=2)
                    nc.vector.tensor_add(
                        out=ostg, in0=ps, in1=bo_bcast[:, nt * 512 : (nt + 1) * 512]
                    )
                    nc.sync.dma_start(
                        out=out[
                            tokt * 128 : (tokt + 1) * 128, nt * 512 : (nt + 1) * 512
                        ],
                        in_=ostg,
                    )

    nc.compile()
    return nc


def kernel(x, wq, bq, wk, bk, wv, bv, wo, bo, trace=False):
    x = np.asarray(x, dtype=np.float32)
    wq = np.asarray(wq, dtype=np.float32)
    bq = np.asarray(bq, dtype=np.float32)
    wk = np.asarray(wk, dtype=np.float32)
    bk = np.asarray(bk, dtype=np.float32)
    wv = np.asarray(wv, dtype=np.float32)
    bv = np.asarray(bv, dtype=np.float32)
    wo = np.asarray(wo, dtype=np.float32)
    bo = np.asarray(bo, dtype=np.float32)

    if "nc" not in _CACHE:
        _CACHE["nc"] = build_nc()
    nc = _CACHE["nc"]

    f16 = np.float16
    wq_t = np.ascontiguousarray(wq.transpose(1, 0, 2).reshape(D, H * E), dtype=f16)
    wk_t = np.ascontiguousarray(wk.transpose(1, 0, 2).reshape(D, H * E), dtype=f16)
    wv_t = np.ascontiguousarray(wv.transpose(1, 0, 2).reshape(D, H * E), dtype=f16)
    wo_t = np.ascontiguousarray(wo.T, dtype=f16)
    bqp = np.ascontiguousarray(bq.reshape(H * E).reshape(8, 128).T)
    bkp = np.ascontiguousarray(bk.reshape(H * E).reshape(8, 128).T)
    bv_row = np.ascontiguousarray(bv.reshape(1, H * E), dtype=f16)
    bo_row = np.ascontiguousarray(bo.reshape(1, D), dtype=f16)

    shared = {
        "wq_t": wq_t,
        "wk_t": wk_t,
        "wv_t": wv_t,
        "wo_t": wo_t,
        "bqp": bqp,
        "bkp": bkp,
        "bv_row": bv_row,
        "bo_row": bo_row,
    }
    in_maps = []
    for c in range(NCORES):
        b, qh = c // 2, c % 2
        xT_c = np.ascontiguousarray(x[b].T, dtype=f16)
        m = dict(shared)
        m["xT"] = xT_c
        m["xTq"] = np.ascontiguousarray(xT_c[:, qh * TQ : (qh + 1) * TQ])
        in_maps.append(m)

    res = run_bass_kernel_spmd(nc, in_maps, list(range(NCORES)), trace=trace)

    out = np.empty((B, S, D), dtype=np.float32)
    for c in range(NCORES):
        b, qh = c // 2, c % 2
        out[b, qh * TQ : (qh + 1) * TQ, :] = res.results[c]["out"]
    if trace:
        return out, res
    return out


# revision 20
# speedup vs baseline: 1.1154x; 1.1154x over previous
"""Multi-head attention (B=4, S=2048, D=1024, H=16, E=64) on 8 TRN2 NeuronCores.

Sharding: core c handles batch b=c//2 and query-half qh=c%2 (1024 query tokens).
K/V are computed per-core for the full 2048-token sequence of its batch (2x
duplicated K/V projection work, but zero collectives / zero cross-core deps).

v2 (vs fp32r baseline at 868us):
  - all PE matmuls in fp16 (1 cyc/row vs fp32_mode=HIGH's ~2, FWL enabled,
    and roughly half the PE energy -> less HAM duty-cycle throttling)
  - scores matmuls for the 2 heads of a pass are row-tiled (head A in PE rows
    0-63, head B in rows 64-127 via tile_position) and issued adjacently so
    they run concurrently in the array
  - exp computed as exp(s/8 - 4) so fp16 can't overflow (max observed |s/8|
    ~7.6; the constant shift cancels in the softmax normalization)
  - V kept fully resident in SBUF (no DRAM spill round-trip)
  - V/out-proj biases applied on DVE from PE-broadcast bias tiles (no
    ones-row matmuls in inner loops)

Per-core program (SPMD, identical on all cores):
  phase 0: V = x @ wv + bv for all 16 heads -> v_all[tok128, chunk, head, 65]
           with a ones-column at [...,64] (softmax sums ride along att@V).
  passes p=0..7 (heads 2p, 2p+1):
    KT[128he, 2048tok] = (wk_p.T @ xT) + bk, QT[128he, 1024tq] likewise
    per tq-tile of 512: per chunk-pair g: scoresT = KT.T-slices @ QT (A,B
    row-tiled), exp on ScalarE from PSUM, attT[65,512] += [V|1].T @ exp
    normalize: recip(sum row) -> ones-matmul broadcast -> DVE multiply
  phase 2: out[tok,1024] = att @ wo.T + bo (bias via DVE add)
"""

import os

import numpy as np

import concourse.bass as bass
import concourse.mybir as mybir
import concourse.tile as tile
from concourse import bacc
from concourse.bass_utils import run_bass_kernel_spmd

FP32 = mybir.dt.float32
FP16 = mybir.dt.float16
AF = mybir.ActivationFunctionType

B, S, D, H, E = 4, 2048, 1024, 16, 64
NCORES = 8
TQ = S // 2  # query tokens per core
SCALE = 1.0 / float(np.sqrt(E))
# exp(s*SCALE + EXP_BIAS): the shift cancels in softmax normalization and
# keeps fp16 exp finite up to s*SCALE = 19 (observed max over this input
# distribution is ~15.2)
EXP_BIAS = -8.0

_CACHE = {}
DEBUG = bool(int(os.environ.get("KDBG", "0")))


def build_nc():
    nc = bacc.Bacc("TRN2", target_bir_lowering=False)

    xT = nc.dram_tensor("xT", [D, S], FP16, kind="ExternalInput")
    xTq = nc.dram_tensor("xTq", [D, TQ], FP16, kind="ExternalInput")
    wq_t = nc.dram_tensor("wq_t", [D, H * E], FP16, kind="ExternalInput")
    wk_t = nc.dram_tensor("wk_t", [D, H * E], FP16, kind="ExternalInput")
    wv_t = nc.dram_tensor("wv_t", [D, H * E], FP16, kind="ExternalInput")
    wo_t = nc.dram_tensor("wo_t", [D, D], FP16, kind="ExternalInput")
    bqp = nc.dram_tensor("bqp", [128, 8], FP32, kind="ExternalInput")
    bkp = nc.dram_tensor("bkp", [128, 8], FP32, kind="ExternalInput")
    bv_row = nc.dram_tensor("bv_row", [1, H * E], FP16, kind="ExternalInput")
    bo_row = nc.dram_tensor("bo_row", [1, D], FP16, kind="ExternalInput")
    out = nc.dram_tensor("out", [TQ, D], FP32, kind="ExternalOutput")
    if DEBUG:
        dbg_v = nc.dram_tensor("dbg_v", [128, 16, H, E + 1], FP16, kind="ExternalOutput")
        dbg_kt = nc.dram_tensor("dbg_kt", [128, S], FP16, kind="ExternalOutput")
        dbg_qt = nc.dram_tensor("dbg_qt", [128, TQ], FP16, kind="ExternalOutput")
        dbg_exp = nc.dram_tensor("dbg_exp", [128, 2, 512], FP16, kind="ExternalOutput")
        dbg_rec = nc.dram_tensor("dbg_rec", [1, 512], FP32, kind="ExternalOutput")
        dbg_att = nc.dram_tensor("dbg_att", [128, TQ], FP16, kind="ExternalOutput")

    xT_r = xT.rearrange("(t p) s -> p t s", p=128)  # [128, 8, 2048]
    xTq_r = xTq.rearrange("(t p) s -> p t s", p=128)  # [128, 8, 1024]
    wq_r = wq_t.rearrange("(t p) m -> p t m", p=128)  # [128, 8, 1024]
    wk_r = wk_t.rearrange("(t p) m -> p t m", p=128)
    wv_r = wv_t.rearrange("(t p) m -> p t m", p=128)
    wo_r = wo_t.rearrange("(t p) m -> p t m", p=128)

    with tile.TileContext(nc) as tc:
        with (
            tc.tile_pool(name="xt", bufs=1) as xt_pool,
            tc.tile_pool(name="wkq", bufs=2) as wkq_pool,
            tc.tile_pool(name="wv", bufs=2) as wv_pool,
            tc.tile_pool(name="wo", bufs=1) as wo_pool,
            tc.tile_pool(name="kt", bufs=2) as kt_pool,
            tc.tile_pool(name="qt", bufs=2) as qt_pool,
            tc.tile_pool(name="vall", bufs=1) as vall_pool,
            tc.tile_pool(name="expp", bufs=4) as exp_pool,
            tc.tile_pool(name="attT", bufs=8) as attT_pool,
            tc.tile_pool(name="small", bufs=2) as small_pool,
            tc.tile_pool(name="ones", bufs=1) as ones_pool,
            tc.tile_pool(name="ps_s", bufs=2, space="PSUM") as ps_scores,
            tc.tile_pool(name="ps_a", bufs=2, space="PSUM") as ps_att,
            tc.tile_pool(name="ps_g", bufs=2, space="PSUM") as ps_gen,
        ):
            # ---- persistent tiles ----
            # DMA order matters: tiny bias rows + the first wv half go first
            # so the PE's broadcast matmuls and V-projection aren't stuck
            # behind the 6MB x upload in the queue.
            xt_sb = xt_pool.tile([128, 8, S], FP16, tag="xt")  # 32KB/part
            xtq_sb = xt_pool.tile([128, 8, TQ], FP16, tag="xtq")  # 16KB/part

            ones_row_f = ones_pool.tile([1, 128], FP32, tag="onesrf")
            nc.vector.memset(ones_row_f, 1.0)
            ones_sb = ones_pool.tile([1, 128], FP16, tag="ones")
            with nc.allow_low_precision(reason="ones constant"):
                nc.vector.tensor_copy(out=ones_sb, in_=ones_row_f)
            bq_sb = ones_pool.tile([128, 8], FP32, tag="bq")
            bk_sb = ones_pool.tile([128, 8], FP32, tag="bk")
            nc.sync.dma_start(out=bq_sb, in_=bqp[:, :])
            nc.sync.dma_start(out=bk_sb, in_=bkp[:, :])
            ebias_sb = ones_pool.tile([128, 1], FP32, tag="ebias")
            nc.vector.memset(ebias_sb, EXP_BIAS)
            bv_sb = ones_pool.tile([1, H * E], FP16, tag="bv")
            bo_sb = ones_pool.tile([1, D], FP16, tag="bo")
            nc.sync.dma_start(out=bv_sb, in_=bv_row[:, :])
            nc.sync.dma_start(out=bo_sb, in_=bo_row[:, :])

            wv_sb0 = wv_pool.tile([128, 8, 512], FP16, tag="wv")
            nc.sync.dma_start(out=wv_sb0, in_=wv_r[:, :, 0:512])
            # split x upload so the first V-proj tiles start sooner
            for sc in range(4):
                nc.sync.dma_start(
                    out=xt_sb[:, :, sc * 512 : (sc + 1) * 512],
                    in_=xT_r[:, :, sc * 512 : (sc + 1) * 512],
                )
            nc.sync.dma_start(out=xtq_sb, in_=xTq_r)
            wv_sb1 = wv_pool.tile([128, 8, 512], FP16, tag="wv")
            nc.sync.dma_start(out=wv_sb1, in_=wv_r[:, :, 512:1024])

            # broadcast bias rows to all 128 partitions (once, via PE ones-matmul)
            bv_bcast = ones_pool.tile([128, H * E], FP32, tag="bvb")  # 4KB
            bo_bcast = ones_pool.tile([128, D], FP32, tag="bob")  # 4KB
            for nt in range(2):
                ps = ps_gen.tile([128, 512], FP32, tag="gen")
                nc.tensor.matmul(
                    out=ps,
                    lhsT=ones_sb[:, :128],
                    rhs=bv_sb[:, nt * 512 : (nt + 1) * 512],
                    start=True,
                    stop=True,
                )
                nc.vector.tensor_copy(
                    out=bv_bcast[:, nt * 512 : (nt + 1) * 512], in_=ps
                )
                ps2 = ps_gen.tile([128, 512], FP32, tag="gen")
                nc.tensor.matmul(
                    out=ps2,
                    lhsT=ones_sb[:, :128],
                    rhs=bo_sb[:, nt * 512 : (nt + 1) * 512],
                    start=True,
                    stop=True,
                )
                nc.vector.tensor_copy(
                    out=bo_bcast[:, nt * 512 : (nt + 1) * 512], in_=ps2
                )

            # V resident in SBUF: [tok128, chunk, head, E+1], ones at [...,E].
            # Full-tile memset (contiguous); V-proj writes then fill [...,:E],
            # leaving the ones column intact.
            v_all = vall_pool.tile([128, 16, H, E + 1], FP16, tag="vall")  # 33KB
            with nc.allow_low_precision(reason="ones column"):
                nc.vector.memset(v_all, 1.0)

            attT_tiles = [
                attT_pool.tile([128, TQ], FP16, tag="attT", name=f"attT{i}")
                for i in range(8)
            ]

            # ---- helpers: PE work groups (emitted inline or as attention fillers) ----
            def v_group(wv_sb, nt, tokt):
                def go():
                    ps = ps_gen.tile([128, 512], FP32, tag="gen")
                    for k in range(8):
                        nc.tensor.matmul(
                            out=ps,
                            lhsT=xt_sb[:, k, tokt * 128 : (tokt + 1) * 128],
                            rhs=wv_sb[:, k, :],
                            start=(k == 0),
                            stop=(k == 7),
                        )
                    with nc.allow_low_precision(reason="V to fp16"):
                        nc.vector.tensor_add(
                            out=v_all[:, tokt, nt * 8 : (nt + 1) * 8, :E],
                            in0=ps.rearrange("p (h e) -> p h e", e=E),
                            in1=bv_bcast[
                                :, nt * 512 : (nt + 1) * 512
                            ].rearrange("p (h e) -> p h e", e=E),
                        )
                return go

            def kt_group(wk_sb, kt_sb, p, ts):
                def go():
                    ps = ps_gen.tile([128, 512], FP32, tag="gen")
                    for k in range(8):
                        nc.tensor.matmul(
                            out=ps,
                            lhsT=wk_sb[:, k, :],
                            rhs=xt_sb[:, k, ts * 512 : (ts + 1) * 512],
                            start=(k == 0),
                            stop=(k == 7),
                        )
                    with nc.allow_low_precision(reason="K to fp16"):
                        nc.vector.tensor_scalar_add(
                            out=kt_sb[:, ts * 512 : (ts + 1) * 512],
                            in0=ps,
                            scalar1=bk_sb[:, p : p + 1],
                        )
                return go

            def qt_group(wq_sb, qt_sb, p, qs):
                def go():
                    ps = ps_gen.tile([128, 512], FP32, tag="gen")
                    for k in range(8):
                        nc.tensor.matmul(
                            out=ps,
                            lhsT=wq_sb[:, k, :],
                            rhs=xtq_sb[:, k, qs * 512 : (qs + 1) * 512],
                            start=(k == 0),
                            stop=(k == 7),
                        )
                    with nc.allow_low_precision(reason="Q to fp16"):
                        nc.vector.tensor_scalar_add(
                            out=qt_sb[:, qs * 512 : (qs + 1) * 512],
                            in0=ps,
                            scalar1=bq_sb[:, p : p + 1],
                        )
                return go

            def out_group(tokt, nt):
                def go():
                    ps = ps_gen.tile([128, 512], FP32, tag="gen")
                    for t in range(8):
                        nc.tensor.matmul(
                            out=ps,
                            lhsT=attT_tiles[t][:, tokt * 128 : (tokt + 1) * 128],
                            rhs=wo_sb[:, nt * 8 + t, :],
                            start=(t == 0),
                            stop=(t == 7),
                        )
                    ostg = small_pool.tile([128, 512], FP32, tag="stg", bufs=2)
                    nc.vector.tensor_add(
                        out=ostg, in0=ps, in1=bo_bcast[:, nt * 512 : (nt + 1) * 512]
                    )
                    nc.sync.dma_start(
                        out=out[
                            tokt * 128 : (tokt + 1) * 128, nt * 512 : (nt + 1) * 512
                        ],
                        in_=ostg,
                    )
                return go

            def ktqt_groups(p):
                """DMA wk/wq for pass p, allocate kt/qt tiles, return proj groups."""
                wk_sb = wkq_pool.tile([128, 8, 128], FP16, tag="wk")
                wq_sb = wkq_pool.tile([128, 8, 128], FP16, tag="wq")
                nc.sync.dma_start(out=wk_sb, in_=wk_r[:, :, p * 128 : (p + 1) * 128])
                nc.sync.dma_start(out=wq_sb, in_=wq_r[:, :, p * 128 : (p + 1) * 128])
                kt_sb = kt_pool.tile([128, S], FP16, tag="kt")
                qt_sb = qt_pool.tile([128, TQ], FP16, tag="qt")
                groups = [kt_group(wk_sb, kt_sb, p, ts) for ts in range(4)]
                groups += [qt_group(wq_sb, qt_sb, p, qs) for qs in range(2)]
                return kt_sb, qt_sb, groups

            # ---- phase 0: V projection nt=0 + pass-0 K/Q, emitted densely ----
            for tokt in range(16):
                v_group(wv_sb0, 0, tokt)()
            kt_sb, qt_sb, g0 = ktqt_groups(0)
            for g in g0:
                g()

            # Fillers keep the PE fed during the ScalarE-paced attention loop.
            # must_do: next pass's K/Q projections (deadline = end of this
            # pass); extra: nt=1 V groups (heads 8-15, deadline = pass 4).
            must_do = []
            extra = [v_group(wv_sb1, 1, tokt) for tokt in range(16)]

            def pop_filler(reserve=0):
                if len(must_do) > reserve:
                    must_do.pop(0)()
                elif extra:
                    extra.pop(0)()

            # ---- passes: 2 heads each (A at partitions 0:64, B at 64:128) ----
            wo_sb = wo_pool.tile([128, 16, 512], FP16, tag="wo")  # 16KB
            for p in range(8):
                if p == 6:
                    for nt in range(2):
                        nc.sync.dma_start(
                            out=wo_sb[:, nt * 8 : (nt + 1) * 8, :],
                            in_=wo_r[:, :, nt * 512 : (nt + 1) * 512],
                        )
                if p < 7:
                    kt_next, qt_next, gnext = ktqt_groups(p + 1)
                    must_do.extend(gnext)
                if p == 4:
                    while extra:  # all V must exist before heads 8-15 attend
                        extra.pop(0)()

                if DEBUG and p == 0:
                    nc.sync.dma_start(out=dbg_kt[:, :], in_=kt_sb)
                    nc.sync.dma_start(out=dbg_qt[:, :], in_=qt_sb)

                hA, hB = 2 * p, 2 * p + 1
                for tqt in range(2):
                    att_psA = ps_att.tile([E + 1, 512], FP32, tag="att")
                    att_psB = ps_att.tile([E + 1, 512], FP32, tag="att")
                    for t in range(16):
                        # one 2-bank PSUM tile holds chunk t's scores for BOTH
                        # heads; the pair runs concurrently in disjoint PE row
                        # groups, and one exp instruction covers both
                        ps_s = ps_scores.tile([128, 2, 512], FP32, tag="sc")
                        nc.tensor.matmul(
                            out=ps_s[:, 0, :],
                            lhsT=kt_sb[0:64, t * 128 : (t + 1) * 128],
                            rhs=qt_sb[0:64, tqt * 512 : (tqt + 1) * 512],
                            start=True,
                            stop=True,
                            tile_position=(0, 0),
                        )
                        nc.tensor.matmul(
                            out=ps_s[:, 1, :],
                            lhsT=kt_sb[64:128, t * 128 : (t + 1) * 128],
                            rhs=qt_sb[64:128, tqt * 512 : (tqt + 1) * 512],
                            start=True,
                            stop=True,
                            tile_position=(64, 0),
                        )
                        exp_t = exp_pool.tile([128, 2, 512], FP16, tag="exp")
                        nc.scalar.activation(
                            out=exp_t, in_=ps_s, func=AF.Exp, scale=SCALE,
                            bias=ebias_sb[:, :],
                        )
                        if DEBUG and p == 0 and tqt == 0 and t == 0:
                            nc.sync.dma_start(out=dbg_exp[:, 0, :], in_=exp_t[:, 0, :])
                        if DEBUG and p == 0 and tqt == 0 and t == 1:
                            nc.sync.dma_start(out=dbg_exp[:, 1, :], in_=exp_t[:, 0, :])
                        nc.tensor.matmul(
                            out=att_psA,
                            lhsT=v_all[:, t, hA, :],
                            rhs=exp_t[:, 0, :],
                            start=(t == 0),
                            stop=(t == 15),
                        )
                        nc.tensor.matmul(
                            out=att_psB,
                            lhsT=v_all[:, t, hB, :],
                            rhs=exp_t[:, 1, :],
                            start=(t == 0),
                            stop=(t == 15),
                        )
                        # keep the PE fed while ScalarE crunches exp
                        if t in (3, 7, 11):
                            pop_filler(reserve=2)
                    # tail fillers: occupy the PE while ScalarE drains the
                    # last exp chunks and DVE computes the softmax recip, so
                    # the rb matmuls / next-tqt scores aren't head-of-line
                    # blocked behind an idle wait
                    pop_filler()
                    pop_filler()
                    for hh, att_ps in ((0, att_psA), (1, att_psB)):
                        base = hh * 64
                        rec32 = small_pool.tile([1, 512], FP32, tag="rec32", bufs=2)
                        nc.vector.reciprocal(out=rec32, in_=att_ps[E : E + 1, :])
                        if DEBUG and p == 0 and tqt == 0 and hh == 0:
                            nc.sync.dma_start(out=dbg_rec[:, :], in_=rec32)
                        recip_r = small_pool.tile(
                            [1, 512], FP16, tag="recr", bufs=2
                        )
                        with nc.allow_low_precision(reason="softmax recip"):
                            nc.vector.tensor_copy(out=recip_r, in_=rec32)
                        rb_ps = ps_gen.tile([64, 512], FP32, tag="gen")
                        nc.tensor.matmul(
                            out=rb_ps,
                            lhsT=ones_sb[:, :64],
                            rhs=recip_r,
                            start=True,
                            stop=True,
                        )
                        rb_sb = small_pool.tile([64, 512], FP32, tag="stg", bufs=2)
                        nc.vector.tensor_copy(out=rb_sb, in_=rb_ps)
                        with nc.allow_low_precision(reason="attT fp16"):
                            nc.vector.tensor_mul(
                                out=attT_tiles[p][
                                    base : base + 64,
                                    tqt * 512 : (tqt + 1) * 512,
                                ],
                                in0=att_ps[:E, :],
                                in1=rb_sb,
                            )

                    if p == 7 and tqt == 0:
                        # out-proj for token chunks 0-3 reads only the tqt=0
                        # half of every attT tile -> overlaps tqt=1 attention
                        must_do.extend(
                            out_group(tokt, nt) for tokt in range(4) for nt in range(2)
                        )

                # next pass's K/Q must be fully emitted before its attention
                while must_do:
                    must_do.pop(0)()
                if p < 7:
                    kt_sb, qt_sb = kt_next, qt_next

            if DEBUG:
                nc.sync.dma_start(out=dbg_att[:, :], in_=attT_tiles[0])

            # ---- phase 2: remaining output projection (tokt 0-3 were
            # emitted during pass 7's second attention half) ----
            for tokt in range(4, 8):
                for nt in range(2):
                    out_group(tokt, nt)()

    nc.compile()
    return nc


def kernel(x, wq, bq, wk, bk, wv, bv, wo, bo, trace=False):
    x = np.asarray(x, dtype=np.float32)
    wq = np.asarray(wq, dtype=np.float32)
    bq = np.asarray(bq, dtype=np.float32)
    wk = np.asarray(wk, dtype=np.float32)
    bk = np.asarray(bk, dtype=np.float32)
    wv = np.asarray(wv, dtype=np.float32)
    bv = np.asarray(bv, dtype=np.float32)
    wo = np.asarray(wo, dtype=np.float32)
    bo = np.asarray(bo, dtype=np.float32)

    if "nc" not in _CACHE:
        _CACHE["nc"] = build_nc()
    nc = _CACHE["nc"]

    f16 = np.float16
    wq_t = np.ascontiguousarray(wq.transpose(1, 0, 2).reshape(D, H * E), dtype=f16)
    wk_t = np.ascontiguousarray(wk.transpose(1, 0, 2).reshape(D, H * E), dtype=f16)
    wv_t = np.ascontiguousarray(wv.transpose(1, 0, 2).reshape(D, H * E), dtype=f16)
    wo_t = np.ascontiguousarray(wo.T, dtype=f16)
    bqp = np.ascontiguousarray(bq.reshape(H * E).reshape(8, 128).T)
    bkp = np.ascontiguousarray(bk.reshape(H * E).reshape(8, 128).T)
    bv_row = np.ascontiguousarray(bv.reshape(1, H * E), dtype=f16)
    bo_row = np.ascontiguousarray(bo.reshape(1, D), dtype=f16)

    shared = {
        "wq_t": wq_t,
        "wk_t": wk_t,
        "wv_t": wv_t,
        "wo_t": wo_t,
        "bqp": bqp,
        "bkp": bkp,
        "bv_row": bv_row,
        "bo_row": bo_row,
    }
    in_maps = []
    for c in range(NCORES):
        b, qh = c // 2, c % 2
        xT_c = np.ascontiguousarray(x[b].T, dtype=f16)
        m = dict(shared)
        m["xT"] = xT_c
        m["xTq"] = np.ascontiguousarray(xT_c[:, qh * TQ : (qh + 1) * TQ])
        in_maps.append(m)

    res = run_bass_kernel_spmd(nc, in_maps, list(range(NCORES)), trace=trace)

    out = np.empty((B, S, D), dtype=np.float32)
    for c in range(NCORES):
        b, qh = c // 2, c % 2
        out[b, qh * TQ : (qh + 1) * TQ, :] = res.results[c]["out"]
    if trace:
        return out, res
    return out


# revision 21
# speedup vs baseline: 1.1447x; 1.0263x over previous
"""Multi-head attention (B=4, S=2048, D=1024, H=16, E=64) on 8 TRN2 NeuronCores.

Sharding: core c handles batch b=c//2 and query-half qh=c%2 (1024 query tokens).
K/V are computed per-core for the full 2048-token sequence of its batch (2x
duplicated K/V projection work, but zero collectives / zero cross-core deps).

v2 (vs fp32r baseline at 868us):
  - all PE matmuls in fp16 (1 cyc/row vs fp32_mode=HIGH's ~2, FWL enabled,
    and roughly half the PE energy -> less HAM duty-cycle throttling)
  - scores matmuls for the 2 heads of a pass are row-tiled (head A in PE rows
    0-63, head B in rows 64-127 via tile_position) and issued adjacently so
    they run concurrently in the array
  - exp computed as exp(s/8 - 4) so fp16 can't overflow (max observed |s/8|
    ~7.6; the constant shift cancels in the softmax normalization)
  - V kept fully resident in SBUF (no DRAM spill round-trip)
  - V/out-proj biases applied on DVE from PE-broadcast bias tiles (no
    ones-row matmuls in inner loops)

Per-core program (SPMD, identical on all cores):
  phase 0: V = x @ wv + bv for all 16 heads -> v_all[tok128, chunk, head, 65]
           with a ones-column at [...,64] (softmax sums ride along att@V).
  passes p=0..7 (heads 2p, 2p+1):
    KT[128he, 2048tok] = (wk_p.T @ xT) + bk, QT[128he, 1024tq] likewise
    per tq-tile of 512: per chunk-pair g: scoresT = KT.T-slices @ QT (A,B
    row-tiled), exp on ScalarE from PSUM, attT[65,512] += [V|1].T @ exp
    normalize: recip(sum row) -> ones-matmul broadcast -> DVE multiply
  phase 2: out[tok,1024] = att @ wo.T + bo (bias via DVE add)
"""

import os

import numpy as np

import concourse.bass as bass
import concourse.mybir as mybir
import concourse.tile as tile
from concourse import bacc
from concourse.bass_utils import run_bass_kernel_spmd

FP32 = mybir.dt.float32
FP16 = mybir.dt.float16
AF = mybir.ActivationFunctionType

B, S, D, H, E = 4, 2048, 1024, 16, 64
NCORES = 8
TQ = S // 2  # query tokens per core
SCALE = 1.0 / float(np.sqrt(E))
# exp(s*SCALE + EXP_BIAS): the shift cancels in softmax normalization and
# keeps fp16 exp finite up to s*SCALE = 19 (observed max over this input
# distribution is ~15.2)
EXP_BIAS = -8.0

_CACHE = {}
DEBUG = bool(int(os.environ.get("KDBG", "0")))


def build_nc():
    nc = bacc.Bacc("TRN2", target_bir_lowering=False)

    xT = nc.dram_tensor("xT", [D, S], FP16, kind="ExternalInput")
    xTq = nc.dram_tensor("xTq", [D, TQ], FP16, kind="ExternalInput")
    wq_t = nc.dram_tensor("wq_t", [D, H * E], FP16, kind="ExternalInput")
    wk_t = nc.dram_tensor("wk_t", [D, H * E], FP16, kind="ExternalInput")
    wv_t = nc.dram_tensor("wv_t", [D, H * E], FP16, kind="ExternalInput")
    wo_t = nc.dram_tensor("wo_t", [D, D], FP16, kind="ExternalInput")
    bqp = nc.dram_tensor("bqp", [128, 8], FP32, kind="ExternalInput")
    bkp = nc.dram_tensor("bkp", [128, 8], FP32, kind="ExternalInput")
    bv_row = nc.dram_tensor("bv_row", [1, H * E], FP16, kind="ExternalInput")
    bo_row = nc.dram_tensor("bo_row", [1, D], FP16, kind="ExternalInput")
    out = nc.dram_tensor("out", [TQ, D], FP32, kind="ExternalOutput")
    if DEBUG:
        dbg_v = nc.dram_tensor("dbg_v", [128, 16, H, E + 1], FP16, kind="ExternalOutput")
        dbg_kt = nc.dram_tensor("dbg_kt", [128, S], FP16, kind="ExternalOutput")
        dbg_qt = nc.dram_tensor("dbg_qt", [128, TQ], FP16, kind="ExternalOutput")
        dbg_exp = nc.dram_tensor("dbg_exp", [128, 2, 512], FP16, kind="ExternalOutput")
        dbg_rec = nc.dram_tensor("dbg_rec", [1, 512], FP32, kind="ExternalOutput")
        dbg_att = nc.dram_tensor("dbg_att", [128, TQ], FP16, kind="ExternalOutput")

    xT_r = xT.rearrange("(t p) s -> p t s", p=128)  # [128, 8, 2048]
    xTq_r = xTq.rearrange("(t p) s -> p t s", p=128)  # [128, 8, 1024]
    wq_r = wq_t.rearrange("(t p) m -> p t m", p=128)  # [128, 8, 1024]
    wk_r = wk_t.rearrange("(t p) m -> p t m", p=128)
    wv_r = wv_t.rearrange("(t p) m -> p t m", p=128)
    wo_r = wo_t.rearrange("(t p) m -> p t m", p=128)

    with tile.TileContext(nc) as tc:
        with (
            tc.tile_pool(name="xt", bufs=1) as xt_pool,
            tc.tile_pool(name="wkq", bufs=2) as wkq_pool,
            tc.tile_pool(name="wv", bufs=2) as wv_pool,
            tc.tile_pool(name="wo", bufs=1) as wo_pool,
            tc.tile_pool(name="kt", bufs=2) as kt_pool,
            tc.tile_pool(name="qt", bufs=2) as qt_pool,
            tc.tile_pool(name="vall", bufs=1) as vall_pool,
            tc.tile_pool(name="expp", bufs=4) as exp_pool,
            tc.tile_pool(name="attT", bufs=8) as attT_pool,
            tc.tile_pool(name="small", bufs=2) as small_pool,
            tc.tile_pool(name="ones", bufs=1) as ones_pool,
            tc.tile_pool(name="ps_s", bufs=2, space="PSUM") as ps_scores,
            tc.tile_pool(name="ps_a", bufs=2, space="PSUM") as ps_att,
            tc.tile_pool(name="ps_g", bufs=2, space="PSUM") as ps_gen,
        ):
            # ---- persistent tiles ----
            # DMA order matters: tiny bias rows + the first wv half go first
            # so the PE's broadcast matmuls and V-projection aren't stuck
            # behind the 6MB x upload in the queue.
            xt_sb = xt_pool.tile([128, 8, S], FP16, tag="xt")  # 32KB/part
            xtq_sb = xt_pool.tile([128, 8, TQ], FP16, tag="xtq")  # 16KB/part

            ones_row_f = ones_pool.tile([1, 128], FP32, tag="onesrf")
            nc.vector.memset(ones_row_f, 1.0)
            ones_sb = ones_pool.tile([1, 128], FP16, tag="ones")
            with nc.allow_low_precision(reason="ones constant"):
                nc.vector.tensor_copy(out=ones_sb, in_=ones_row_f)
            bq_sb = ones_pool.tile([128, 8], FP32, tag="bq")
            bk_sb = ones_pool.tile([128, 8], FP32, tag="bk")
            nc.sync.dma_start(out=bq_sb, in_=bqp[:, :])
            nc.sync.dma_start(out=bk_sb, in_=bkp[:, :])
            ebias_sb = ones_pool.tile([128, 1], FP32, tag="ebias")
            nc.vector.memset(ebias_sb, EXP_BIAS)
            bv_sb = ones_pool.tile([1, H * E], FP16, tag="bv")
            bo_sb = ones_pool.tile([1, D], FP16, tag="bo")
            nc.sync.dma_start(out=bv_sb, in_=bv_row[:, :])
            nc.sync.dma_start(out=bo_sb, in_=bo_row[:, :])

            wv_sb0 = wv_pool.tile([128, 8, 512], FP16, tag="wv")
            nc.sync.dma_start(out=wv_sb0, in_=wv_r[:, :, 0:512])
            # split x upload so the first V-proj tiles start sooner
            for sc in range(4):
                nc.sync.dma_start(
                    out=xt_sb[:, :, sc * 512 : (sc + 1) * 512],
                    in_=xT_r[:, :, sc * 512 : (sc + 1) * 512],
                )
            nc.sync.dma_start(out=xtq_sb, in_=xTq_r)
            wv_sb1 = wv_pool.tile([128, 8, 512], FP16, tag="wv")
            nc.sync.dma_start(out=wv_sb1, in_=wv_r[:, :, 512:1024])

            # broadcast bias rows to all 128 partitions (once, via PE ones-matmul)
            bv_bcast = ones_pool.tile([128, H * E], FP32, tag="bvb")  # 4KB
            bo_bcast = ones_pool.tile([128, D], FP32, tag="bob")  # 4KB
            for nt in range(2):
                ps = ps_gen.tile([128, 512], FP32, tag="gen")
                nc.tensor.matmul(
                    out=ps,
                    lhsT=ones_sb[:, :128],
                    rhs=bv_sb[:, nt * 512 : (nt + 1) * 512],
                    start=True,
                    stop=True,
                )
                nc.vector.tensor_copy(
                    out=bv_bcast[:, nt * 512 : (nt + 1) * 512], in_=ps
                )
                ps2 = ps_gen.tile([128, 512], FP32, tag="gen")
                nc.tensor.matmul(
                    out=ps2,
                    lhsT=ones_sb[:, :128],
                    rhs=bo_sb[:, nt * 512 : (nt + 1) * 512],
                    start=True,
                    stop=True,
                )
                nc.vector.tensor_copy(
                    out=bo_bcast[:, nt * 512 : (nt + 1) * 512], in_=ps2
                )

            # V resident in SBUF: [tok128, chunk, head, E+1], ones at [...,E]
            # (only the strided ones column is memset -- a full-tile memset is
            # ~17us of DVE that stalls the first V-projection drains)
            v_all = vall_pool.tile([128, 16, H, E + 1], FP16, tag="vall")  # 33KB
            with nc.allow_low_precision(reason="ones column"):
                nc.vector.memset(v_all[:, :, :, E : E + 1], 1.0)

            attT_tiles = [
                attT_pool.tile([128, TQ], FP16, tag="attT", name=f"attT{i}")
                for i in range(8)
            ]

            # ---- helpers: PE work groups (emitted inline or as attention fillers) ----
            def v_group(wv_sb, nt, tokt):
                def go():
                    ps = ps_gen.tile([128, 512], FP32, tag="gen")
                    for k in range(8):
                        nc.tensor.matmul(
                            out=ps,
                            lhsT=xt_sb[:, k, tokt * 128 : (tokt + 1) * 128],
                            rhs=wv_sb[:, k, :],
                            start=(k == 0),
                            stop=(k == 7),
                        )
                    with nc.allow_low_precision(reason="V to fp16"):
                        nc.vector.tensor_add(
                            out=v_all[:, tokt, nt * 8 : (nt + 1) * 8, :E],
                            in0=ps.rearrange("p (h e) -> p h e", e=E),
                            in1=bv_bcast[
                                :, nt * 512 : (nt + 1) * 512
                            ].rearrange("p (h e) -> p h e", e=E),
                        )
                return go

            def kt_group(wk_sb, kt_sb, p, ts):
                def go():
                    ps = ps_gen.tile([128, 512], FP32, tag="gen")
                    for k in range(8):
                        nc.tensor.matmul(
                            out=ps,
                            lhsT=wk_sb[:, k, :],
                            rhs=xt_sb[:, k, ts * 512 : (ts + 1) * 512],
                            start=(k == 0),
                            stop=(k == 7),
                        )
                    with nc.allow_low_precision(reason="K to fp16"):
                        nc.vector.tensor_scalar_add(
                            out=kt_sb[:, ts * 512 : (ts + 1) * 512],
                            in0=ps,
                            scalar1=bk_sb[:, p : p + 1],
                        )
                return go

            def qt_group(wq_sb, qt_sb, p, qs):
                def go():
                    ps = ps_gen.tile([128, 512], FP32, tag="gen")
                    for k in range(8):
                        nc.tensor.matmul(
                            out=ps,
                            lhsT=wq_sb[:, k, :],
                            rhs=xtq_sb[:, k, qs * 512 : (qs + 1) * 512],
                            start=(k == 0),
                            stop=(k == 7),
                        )
                    with nc.allow_low_precision(reason="Q to fp16"):
                        nc.vector.tensor_scalar_add(
                            out=qt_sb[:, qs * 512 : (qs + 1) * 512],
                            in0=ps,
                            scalar1=bq_sb[:, p : p + 1],
                        )
                return go

            def out_group(tokt, nt):
                def go():
                    ps = ps_gen.tile([128, 512], FP32, tag="gen")
                    for t in range(8):
                        nc.tensor.matmul(
                            out=ps,
                            lhsT=attT_tiles[t][:, tokt * 128 : (tokt + 1) * 128],
                            rhs=wo_sb[:, nt * 8 + t, :],
                            start=(t == 0),
                            stop=(t == 7),
                        )
                    ostg = small_pool.tile([128, 512], FP32, tag="stg", bufs=2)
                    nc.vector.tensor_add(
                        out=ostg, in0=ps, in1=bo_bcast[:, nt * 512 : (nt + 1) * 512]
                    )
                    nc.sync.dma_start(
                        out=out[
                            tokt * 128 : (tokt + 1) * 128, nt * 512 : (nt + 1) * 512
                        ],
                        in_=ostg,
                    )
                return go

            def ktqt_groups(p):
                """DMA wk/wq for pass p, allocate kt/qt tiles, return proj groups."""
                wk_sb = wkq_pool.tile([128, 8, 128], FP16, tag="wk")
                wq_sb = wkq_pool.tile([128, 8, 128], FP16, tag="wq")
                nc.sync.dma_start(out=wk_sb, in_=wk_r[:, :, p * 128 : (p + 1) * 128])
                nc.sync.dma_start(out=wq_sb, in_=wq_r[:, :, p * 128 : (p + 1) * 128])
                kt_sb = kt_pool.tile([128, S], FP16, tag="kt")
                qt_sb = qt_pool.tile([128, TQ], FP16, tag="qt")
                groups = [kt_group(wk_sb, kt_sb, p, ts) for ts in range(4)]
                groups += [qt_group(wq_sb, qt_sb, p, qs) for qs in range(2)]
                return kt_sb, qt_sb, groups

            # ---- phase 0: V projection nt=0 + pass-0 K/Q, emitted densely ----
            for tokt in range(16):
                v_group(wv_sb0, 0, tokt)()
            kt_sb, qt_sb, g0 = ktqt_groups(0)
            for g in g0:
                g()

            # Fillers keep the PE fed during the ScalarE-paced attention loop.
            # must_do: next pass's K/Q projections (deadline = end of this
            # pass); extra: nt=1 V groups (heads 8-15, deadline = pass 4).
            must_do = []
            extra = [v_group(wv_sb1, 1, tokt) for tokt in range(16)]

            def pop_filler(reserve=0):
                if len(must_do) > reserve:
                    must_do.pop(0)()
                elif extra:
                    extra.pop(0)()

            # ---- passes: 2 heads each (A at partitions 0:64, B at 64:128) ----
            wo_sb = wo_pool.tile([128, 16, 512], FP16, tag="wo")  # 16KB
            for p in range(8):
                if p == 6:
                    for nt in range(2):
                        nc.sync.dma_start(
                            out=wo_sb[:, nt * 8 : (nt + 1) * 8, :],
                            in_=wo_r[:, :, nt * 512 : (nt + 1) * 512],
                        )
                if p < 7:
                    kt_next, qt_next, gnext = ktqt_groups(p + 1)
                    must_do.extend(gnext)
                if p == 4:
                    while extra:  # all V must exist before heads 8-15 attend
                        extra.pop(0)()

                if DEBUG and p == 0:
                    nc.sync.dma_start(out=dbg_kt[:, :], in_=kt_sb)
                    nc.sync.dma_start(out=dbg_qt[:, :], in_=qt_sb)

                hA, hB = 2 * p, 2 * p + 1
                for tqt in range(2):
                    att_psA = ps_att.tile([E + 1, 512], FP32, tag="att")
                    att_psB = ps_att.tile([E + 1, 512], FP32, tag="att")
                    for t in range(16):
                        # one 2-bank PSUM tile holds chunk t's scores for BOTH
                        # heads; the pair runs concurrently in disjoint PE row
                        # groups, and one exp instruction covers both
                        ps_s = ps_scores.tile([128, 2, 512], FP32, tag="sc")
                        nc.tensor.matmul(
                            out=ps_s[:, 0, :],
                            lhsT=kt_sb[0:64, t * 128 : (t + 1) * 128],
                            rhs=qt_sb[0:64, tqt * 512 : (tqt + 1) * 512],
                            start=True,
                            stop=True,
                            tile_position=(0, 0),
                        )
                        nc.tensor.matmul(
                            out=ps_s[:, 1, :],
                            lhsT=kt_sb[64:128, t * 128 : (t + 1) * 128],
                            rhs=qt_sb[64:128, tqt * 512 : (tqt + 1) * 512],
                            start=True,
                            stop=True,
                            tile_position=(64, 0),
                        )
                        exp_t = exp_pool.tile([128, 2, 512], FP16, tag="exp")
                        nc.scalar.activation(
                            out=exp_t, in_=ps_s, func=AF.Exp, scale=SCALE,
                            bias=ebias_sb[:, :],
                        )
                        if DEBUG and p == 0 and tqt == 0 and t == 0:
                            nc.sync.dma_start(out=dbg_exp[:, 0, :], in_=exp_t[:, 0, :])
                        if DEBUG and p == 0 and tqt == 0 and t == 1:
                            nc.sync.dma_start(out=dbg_exp[:, 1, :], in_=exp_t[:, 0, :])
                        nc.tensor.matmul(
                            out=att_psA,
                            lhsT=v_all[:, t, hA, :],
                            rhs=exp_t[:, 0, :],
                            start=(t == 0),
                            stop=(t == 15),
                        )
                        nc.tensor.matmul(
                            out=att_psB,
                            lhsT=v_all[:, t, hB, :],
                            rhs=exp_t[:, 1, :],
                            start=(t == 0),
                            stop=(t == 15),
                        )
                        # keep the PE fed while ScalarE crunches exp (the
                        # high reserve keeps groups back for the tqt tails)
                        if t in (5, 11):
                            pop_filler(reserve=4)
                    # tail fillers: occupy the PE while ScalarE drains the
                    # last exp chunks and DVE computes the softmax recip, so
                    # the rb matmuls / next-tqt scores aren't head-of-line
                    # blocked behind an idle wait
                    pop_filler()
                    pop_filler()
                    for hh, att_ps in ((0, att_psA), (1, att_psB)):
                        base = hh * 64
                        rec32 = small_pool.tile([1, 512], FP32, tag="rec32", bufs=2)
                        nc.vector.reciprocal(out=rec32, in_=att_ps[E : E + 1, :])
                        if DEBUG and p == 0 and tqt == 0 and hh == 0:
                            nc.sync.dma_start(out=dbg_rec[:, :], in_=rec32)
                        recip_r = small_pool.tile(
                            [1, 512], FP16, tag="recr", bufs=2
                        )
                        with nc.allow_low_precision(reason="softmax recip"):
                            nc.vector.tensor_copy(out=recip_r, in_=rec32)
                        rb_ps = ps_gen.tile([64, 512], FP32, tag="gen")
                        nc.tensor.matmul(
                            out=rb_ps,
                            lhsT=ones_sb[:, :64],
                            rhs=recip_r,
                            start=True,
                            stop=True,
                        )
                        rb_sb = small_pool.tile([64, 512], FP32, tag="stg", bufs=2)
                        nc.vector.tensor_copy(out=rb_sb, in_=rb_ps)
                        with nc.allow_low_precision(reason="attT fp16"):
                            nc.vector.tensor_mul(
                                out=attT_tiles[p][
                                    base : base + 64,
                                    tqt * 512 : (tqt + 1) * 512,
                                ],
                                in0=att_ps[:E, :],
                                in1=rb_sb,
                            )

                    if p == 7 and tqt == 0:
                        # out-proj for token chunks 0-3 reads only the tqt=0
                        # half of every attT tile -> overlaps tqt=1 attention
                        must_do.extend(
                            out_group(tokt, nt) for tokt in range(4) for nt in range(2)
                        )

                # next pass's K/Q must be fully emitted before its attention
                while must_do:
                    must_do.pop(0)()
                if p < 7:
                    kt_sb, qt_sb = kt_next, qt_next

            if DEBUG:
                nc.sync.dma_start(out=dbg_att[:, :], in_=attT_tiles[0])

            # ---- phase 2: remaining output projection (tokt 0-3 were
            # emitted during pass 7's second attention half) ----
            for tokt in range(4, 8):
                for nt in range(2):
                    out_group(tokt, nt)()

    nc.compile()
    return nc


def kernel(x, wq, bq, wk, bk, wv, bv, wo, bo, trace=False):
    x = np.asarray(x, dtype=np.float32)
    wq = np.asarray(wq, dtype=np.float32)
    bq = np.asarray(bq, dtype=np.float32)
    wk = np.asarray(wk, dtype=np.float32)
    bk = np.asarray(bk, dtype=np.float32)
    wv = np.asarray(wv, dtype=np.float32)
    bv = np.asarray(bv, dtype=np.float32)
    wo = np.asarray(wo, dtype=np.float32)
    bo = np.asarray(bo, dtype=np.float32)

    if "nc" not in _CACHE:
        _CACHE["nc"] = build_nc()
    nc = _CACHE["nc"]

    f16 = np.float16
    wq_t = np.ascontiguousarray(wq.transpose(1, 0, 2).reshape(D, H * E), dtype=f16)
    wk_t = np.ascontiguousarray(wk.transpose(1, 0, 2).reshape(D, H * E), dtype=f16)
    wv_t = np.ascontiguousarray(wv.transpose(1, 0, 2).reshape(D, H * E), dtype=f16)
    wo_t = np.ascontiguousarray(wo.T, dtype=f16)
    bqp = np.ascontiguousarray(bq.reshape(H * E).reshape(8, 128).T)
    bkp = np.ascontiguousarray(bk.reshape(H * E).reshape(8, 128).T)
    bv_row = np.ascontiguousarray(bv.reshape(1, H * E), dtype=f16)
    bo_row = np.ascontiguousarray(bo.reshape(1, D), dtype=f16)

    shared = {
        "wq_t": wq_t,
        "wk_t": wk_t,
        "wv_t": wv_t,
        "wo_t": wo_t,
        "bqp": bqp,
        "bkp": bkp,
        "bv_row": bv_row,
        "bo_row": bo_row,
    }
    in_maps = []
    for c in range(NCORES):
        b, qh = c // 2, c % 2
        xT_c = np.ascontiguousarray(x[b].T, dtype=f16)
        m = dict(shared)
        m["xT"] = xT_c
        m["xTq"] = np.ascontiguousarray(xT_c[:, qh * TQ : (qh + 1) * TQ])
        in_maps.append(m)

    res = run_bass_kernel_spmd(nc, in_maps, list(range(NCORES)), trace=trace)

    out = np.empty((B, S, D), dtype=np.float32)
    for c in range(NCORES):
        b, qh = c // 2, c % 2
        out[b, qh * TQ : (qh + 1) * TQ, :] = res.results[c]["out"]
    if trace:
        return out, res
    return out
